# revision 12
# baseline (speedup 1.0000x reference)
"""Trainium2 Bass kernel for a single-step attention decoder (DecoderRNN).

Computation (batch=1, single decode step):
  embedded = emb[word]                                  [E]
  x  = concat(last_context, embedded)                   [H+E]
  gates = W_ih @ x + b_ih + W_hh @ h0 + b_hh            [4H] (i,f,g,o)
  c  = sig(f)*c0 + sig(i)*tanh(g);  ht = sig(o)*tanh(c) [H]
  scores = tanh(enc @ Wa_e.T + (Wa_h @ ht + b_attn)) @ v  [S]
  w  = softmax(scores);  context = w @ enc              [E]
  ht_tilda = tanh(W_ah @ concat(context, ht) + b_ah)    [H]
  out = log_softmax(W_out @ ht_tilda + b_out)           [V]

Sharding across 8 NeuronCores, built to minimize the serial chain (three tiny
collectives total, no strided gathers):
  - LSTM rows hidden-sharded: core k produces ht_k, c_k [128].
  - Wa_h and W_ah[:,H:] are COLUMN-sharded: each core turns its own ht_k into
    full-length partial vectors u^k = Wa_h[:,k]@ht_k and w2^k = W_ah[:,H+k]@ht_k;
    AllReduce#1 sums both (payload [128,16], K-layout, contiguous).
  - encoder_out sequence-sharded for scores; exp(scores) partials give an
    unnormalized context partial [128,8] + local softmax normalizer;
    AllReduce#2 sums them (payload [128,9]).
  - ht_tilda computed FULLY on every core (replicated W_ah[:,:H] @ context
    + the AllReduced w2) -> feeds the vocab-sharded W_out matvec directly.
  - log_softmax normalizer via AllGather#3 of per-core sum(exp(logits)) (8 B).
Weight matvecs run on the PE with host-pre-transposed (optionally bf16)
weights; no max-subtraction needed (logits are O(1) by construction).
"""
import numpy as np

import concourse.bass as bass
import concourse.mybir as mybir
import concourse.tile as tile
from concourse import bacc, bass_utils
from concourse.bass_interp import get_hw_module

NCORES = 8
V, E, H, S = 32000, 1024, 1024, 2048
HC = H // NCORES      # 128  hidden chunk per core
SC = S // NCORES      # 256  sequence chunk per core
VC = V // NCORES      # 4000 vocab rows per core
VP = 4096             # padded vocab shard (32 tiles of 128)
KX = (E + H) // 128   # 16   contraction chunks for x
KH = H // 128         # 8    contraction chunks for ht-sized vectors
NEG_BIG = -1.0e5      # pad-row bias: exp() underflows to exactly 0

f32 = mybir.dt.float32
bf16 = mybir.dt.bfloat16
AF = mybir.ActivationFunctionType
ALU = mybir.AluOpType

# dtype config for the heavy weight streams (host casts to match)
WIH_DT = f32    # W_ih (gates)
WO_DT = bf16    # W_out (vocab projection)
WAHC_DT = f32   # W_ah[:, :H] (context part, replicated)
WSM_DT = f32    # Wa_h / W_ah[:,H:] column shards (small)
ATT_DT = f32    # Wa_e / enc (scores + context path stays f32 for w precision)

_G = {}  # build-scoped globals (ones tiles)


def _np_dt(dt):
    return np.float32 if dt == f32 else np.dtype("bfloat16")


# ---------------------------------------------------------------- device code
def _emit_iter(nc, tc, I, O, pools, stop_after=None):
    (wihp, wattp, waep, enctp, encnp, tzp, wahcp, wop, smp, dram, ps) = pools

    def dma(dst, src):
        nc.sync.dma_start(dst, src)

    ones = _G["ones"]        # [128,1] f32 1.0
    one1 = _G["one1"]        # [1,1] WIH_DT 1.0

    # ---- small constants ----
    xk = smp.tile([128, KX], WIH_DT, tag="xk")
    dma(xk[:], I["xk"])
    hbr = smp.tile([1, 512], WIH_DT, tag="hbr")
    dma(hbr[:], I["hbiasr"])
    c0 = smp.tile([128, 1], f32, tag="c0")
    dma(c0[:], I["c0k"])
    ub8 = smp.tile([128, KH], f32, tag="ub8")   # b_attn in K-layout
    dma(ub8[:], I["battn8"])
    bah8 = smp.tile([128, KH], f32, tag="bah8")
    dma(bah8[:], I["bah8"])
    vk = smp.tile([128, KH], f32, tag="vk")
    dma(vk[:], I["vk"])
    bout = smp.tile([128, 32], f32, tag="bout")
    dma(bout[:], I["bout"])

    # ================= Stage A: LSTM gates -> ht_k, c_k =================
    # gate columns: 0=i, 1=f, 2=o, 3=g (host reordered)
    wih = []
    for kc in range(KX):
        t = wihp.tile([128, 512], WIH_DT, tag=f"wih{kc % 8}", bufs=2)
        dma(t[:], I["wihT"][kc * 128:(kc + 1) * 128, :])
        wih.append(t)
    ps_g = ps.tile([128, 4], f32, tag="mm", bufs=3)
    for g in range(4):
        nc.tensor.matmul(ps_g[:, g:g + 1], hbr[:, g * 128:(g + 1) * 128],
                         one1[:], start=True, stop=False)
        for kc in range(KX):
            nc.tensor.matmul(ps_g[:, g:g + 1], wih[kc][:, g * 128:(g + 1) * 128],
                             xk[:, kc:kc + 1], start=False, stop=(kc == KX - 1))
    # sigmoid(x) = 0.5*tanh(0.5x)+0.5 for i,f,o; tanh for g — one table set.
    th3 = smp.tile([128, 3], f32, tag="th3")
    nc.scalar.activation(th3[:], ps_g[:, 0:3], AF.Tanh, scale=0.5)
    tan_g = smp.tile([128, 1], f32, tag="tang")
    nc.scalar.activation(tan_g[:], ps_g[:, 3:4], AF.Tanh)
    sg3 = smp.tile([128, 3], f32, tag="sg3")
    nc.vector.tensor_scalar(sg3[:], th3[:], 1.0, 0.5, ALU.add, ALU.mult)
    t1 = smp.tile([128, 1], f32, tag="sE")
    nc.vector.tensor_mul(t1[:], sg3[:, 1:2], c0[:])      # sig_f * c0
    t2 = smp.tile([128, 1], f32, tag="sF")
    nc.vector.tensor_mul(t2[:], sg3[:, 0:1], tan_g[:])   # sig_i * tanh(g)
    c_sb = smp.tile([128, 1], f32, tag="sG")
    nc.vector.tensor_add(c_sb[:], t1[:], t2[:])
    tan_c = smp.tile([128, 1], f32, tag="sH")
    nc.scalar.activation(tan_c[:], c_sb[:], AF.Tanh)
    ht_sb = smp.tile([128, 1], f32, tag="sI")
    nc.vector.tensor_mul(ht_sb[:], sg3[:, 2:3], tan_c[:])  # sig_o * tanh(c)
    dma(O["hout"], ht_sb[:])
    dma(O["cout"], c_sb[:])
    if stop_after == "A":
        return

    # ===== Stage B: partial u^k, w2^k (column-sharded matvecs) + AR1 =====
    htb = smp.tile([128, 1], WSM_DT, tag="htb")
    nc.vector.tensor_copy(htb[:], ht_sb[:])
    watt = wattp.tile([128, 2048], WSM_DT, tag="watt")   # [Wa_h col | W_ah col]
    dma(watt[:], I["wattT"])
    ps_uw = ps.tile([128, 16], f32, tag="mm", bufs=3)
    for mt in range(16):
        nc.tensor.matmul(ps_uw[:, mt:mt + 1], watt[:, mt * 128:(mt + 1) * 128],
                         htb[:], start=True, stop=True)
    uw_sb = smp.tile([128, 16], f32, tag="uwsb")
    nc.scalar.activation(uw_sb[:], ps_uw[:], AF.Copy)
    ar1_i = dram.tile([128, 16], f32, tag="ar1i")
    ar1_o = dram.tile([128, 16], f32, tag="ar1o")
    dma(ar1_i[:], uw_sb[:])
    nc.gpsimd.collective_compute("AllReduce", ALU.add,
                                 replica_groups=[list(range(NCORES))],
                                 ins=[ar1_i[:]], outs=[ar1_o[:]])
    uw8 = smp.tile([128, 16], f32, tag="uw8")
    dma(uw8[:], ar1_o[:])
    # u8 = u + b_attn (K-layout [128,8])
    u8 = smp.tile([128, KH], f32, tag="u8")
    nc.vector.tensor_add(u8[:], uw8[:, 0:KH], ub8[:])
    if stop_after == "B":
        dma(O["httout"], u8[:])
        return

    # ====== Stage C: Z.T = Wa_e @ enc_k.T ; scores_k = tanh(Z.T+u) @ v ==
    wae, enct = [], []
    for ec in range(KH):
        t = waep.tile([128, H], ATT_DT, tag=f"wae{ec}")
        dma(t[:], I["waeT"][ec * 128:(ec + 1) * 128, :])
        wae.append(t)
        t2_ = enctp.tile([128, SC], ATT_DT, tag=f"enct{ec}")
        dma(t2_[:], I["encT"][ec * 128:(ec + 1) * 128, :])
        enct.append(t2_)
    tz = []
    for hc in range(KH):
        ps_zt = ps.tile([128, SC], f32, tag="zt", bufs=2)
        for ec in range(KH):
            nc.tensor.matmul(ps_zt[:], wae[ec][:, hc * 128:(hc + 1) * 128],
                             enct[ec][:], start=(ec == 0), stop=(ec == KH - 1))
        t = tzp.tile([128, SC], f32, tag=f"tz{hc}")
        nc.scalar.activation(t[:], ps_zt[:], AF.Tanh, bias=u8[:, hc:hc + 1])
        tz.append(t)
    ps_s = ps.tile([128, 2], f32, tag="mm", bufs=3)
    for j in range(SC // 128):
        for hc in range(KH):
            nc.tensor.matmul(ps_s[:, j:j + 1], tz[hc][:, j * 128:(j + 1) * 128],
                             vk[:, hc:hc + 1], start=(hc == 0), stop=(hc == KH - 1))
    exp_sc = smp.tile([128, 2], f32, tag="expsc")
    zrow = smp.tile([128, 1], f32, tag="zrow")
    nc.scalar.activation(exp_sc[:], ps_s[:], AF.Exp, accum_out=zrow[:])

    # ===== Stage D: unnormalized context partial [128,8] + z ; AR2 =====
    encn = []
    for j in range(SC // 128):
        t = encnp.tile([128, E], ATT_DT, tag=f"encn{j}")
        dma(t[:], I["encN"][j * 128:(j + 1) * 128, :])
        encn.append(t)
    ps_ec = ps.tile([128, KH], f32, tag="mm", bufs=3)
    for mt in range(KH):
        for j in range(SC // 128):
            nc.tensor.matmul(ps_ec[:, mt:mt + 1],
                             encn[j][:, mt * 128:(mt + 1) * 128],
                             exp_sc[:, j:j + 1],
                             start=(j == 0), stop=(j == SC // 128 - 1))
    ps_z1 = ps.tile([1, 1], f32, tag="ec", bufs=2)
    nc.tensor.matmul(ps_z1[:], zrow[:], ones[:], start=True, stop=True)
    ecz = smp.tile([128, 9], f32, tag="ecz")
    nc.vector.memset(ecz[:, 8:9], 0.0)
    nc.scalar.activation(ecz[:, 0:8], ps_ec[:], AF.Copy)
    nc.scalar.activation(ecz[0:1, 8:9], ps_z1[:], AF.Copy)
    ar2_i = dram.tile([128, 9], f32, tag="ar2i")
    ar2_o = dram.tile([128, 9], f32, tag="ar2o")
    dma(ar2_i[:], ecz[:])
    nc.gpsimd.collective_compute("AllReduce", ALU.add,
                                 replica_groups=[list(range(NCORES))],
                                 ins=[ar2_i[:]], outs=[ar2_o[:]])
    ectx8 = smp.tile([128, KH], f32, tag="ectx8")
    dma(ectx8[:], ar2_o[:, 0:8])
    zb = smp.tile([128, 1], f32, tag="zb")
    dma(zb[:], ar2_o[0:1, 8:9].to_broadcast((128, 1)))
    rzb = smp.tile([128, 1], f32, tag="rzb")
    nc.vector.reciprocal(rzb[:], zb[:])
    ctx8 = smp.tile([128, KH], WAHC_DT, tag="ctx8")
    nc.vector.tensor_scalar_mul(ctx8[:], ectx8[:], rzb[:])
    w_sb = smp.tile([128, 2], f32, tag="wsb")
    nc.vector.tensor_scalar_mul(w_sb[:], exp_sc[:], rzb[:])
    dma(O["wout"], w_sb[:])
    if stop_after == "C":
        dma(O["httout"], ectx8[:])
        return

    # ===== Stage E: full ht_tilda = tanh(Wah_c@ctx + w2 + b_ah) [128,8] ==
    wahc = []
    for kc in range(KH):
        t = wahcp.tile([128, H], WAHC_DT, tag=f"wahc{kc}")
        dma(t[:], I["wahcT"][kc * 128:(kc + 1) * 128, :])
        wahc.append(t)
    ps_ht = ps.tile([128, KH], f32, tag="mm", bufs=3)
    for mt in range(KH):
        for kc in range(KH):
            nc.tensor.matmul(ps_ht[:, mt:mt + 1],
                             wahc[kc][:, mt * 128:(mt + 1) * 128],
                             ctx8[:, kc:kc + 1],
                             start=(kc == 0), stop=(kc == KH - 1))
    w2b = smp.tile([128, KH], f32, tag="w2b")
    nc.vector.tensor_add(w2b[:], uw8[:, KH:16], bah8[:])
    htsum = smp.tile([128, KH], f32, tag="htsum")
    nc.vector.tensor_add(htsum[:], ps_ht[:], w2b[:])
    htt8 = smp.tile([128, KH], WO_DT, tag="htt8")
    nc.scalar.activation(htt8[:], htsum[:], AF.Tanh)
    htt8f = smp.tile([128, KH], f32, tag="htt8f")
    nc.scalar.activation(htt8f[:], htsum[:], AF.Tanh)
    dma(O["httout"], htt8f[:])
    if stop_after == "E":
        return

    # ====== Stage F: logits_k = W_out[shard] @ ht_tilda ; log_softmax ===
    ps_lg = ps.tile([128, 32], f32, tag="mm", bufs=3)
    for g in range(8):
        wo = wop.tile([128, 4096], WO_DT, tag="wo")
        dma(wo[:], I["woutP"][g, :, :])
        for sub in range(4):
            mc = g * 4 + sub
            for ec in range(KH):
                nc.tensor.matmul(ps_lg[:, mc:mc + 1],
                                 wo[:, sub * 1024 + ec * 128: sub * 1024 + (ec + 1) * 128],
                                 htt8[:, ec:ec + 1],
                                 start=(ec == 0), stop=(ec == KH - 1))
    lg_sb = smp.tile([128, 32], f32, tag="lgsb")
    nc.vector.tensor_add(lg_sb[:], ps_lg[:], bout[:])
    exp2 = smp.tile([128, 32], f32, tag="exp2")
    z2row = smp.tile([128, 1], f32, tag="z2row")
    nc.scalar.activation(exp2[:], lg_sb[:], AF.Exp, accum_out=z2row[:])
    ps_z2 = ps.tile([1, 1], f32, tag="ec", bufs=2)
    nc.tensor.matmul(ps_z2[:], z2row[:], ones[:], start=True, stop=True)
    z2_sb = smp.tile([1, 1], f32, tag="z2sb")
    nc.scalar.activation(z2_sb[:], ps_z2[:], AF.Copy)
    ag_z_i = dram.tile([1, 1], f32, tag="agzi")
    ag_z_o = dram.tile([NCORES, 1], f32, tag="agzo")
    dma(ag_z_i[:], z2_sb[:])
    nc.gpsimd.collective_compute("AllGather", ALU.bypass,
                                 replica_groups=[list(range(NCORES))],
                                 ins=[ag_z_i[:]], outs=[ag_z_o[:]])
    z2all = smp.tile([128, NCORES], f32, tag="z2all")
    dma(z2all[:], ag_z_o[:].rearrange("(x j) y -> x (j y)", x=1).to_broadcast((128, NCORES)))
    z2g = smp.tile([128, 1], f32, tag="z2g")
    nc.vector.reduce_sum(z2g[:], z2all[:], axis=mybir.AxisListType.X)
    lnz = smp.tile([128, 1], f32, tag="lnz")
    nc.scalar.activation(lnz[:], z2g[:], AF.Ln)
    outp = smp.tile([128, 32], f32, tag="outp")
    nc.vector.tensor_scalar(outp[:], lg_sb[:], lnz[:], None, ALU.subtract)
    dma(O["lpout"], outp[:])


def build_module(n_iters=1, wo_bufs=7, stop_after=None):
    """Build + compile the SPMD module."""
    nc = bacc.Bacc("TRN2", target_bir_lowering=False, debug=False,
                   enable_asserts=False, num_devices=NCORES)
    I = {
        "xk":     nc.dram_tensor("xk", [128, KX], WIH_DT, kind="ExternalInput").ap(),
        "hbiasr": nc.dram_tensor("hbiasr", [1, 512], WIH_DT, kind="ExternalInput").ap(),
        "c0k":    nc.dram_tensor("c0k", [128, 1], f32, kind="ExternalInput").ap(),
        "wihT":   nc.dram_tensor("wihT", [E + H, 4 * 128], WIH_DT, kind="ExternalInput").ap(),
        "wattT":  nc.dram_tensor("wattT", [128, 2048], WSM_DT, kind="ExternalInput").ap(),
        "battn8": nc.dram_tensor("battn8", [128, KH], f32, kind="ExternalInput").ap(),
        "waeT":   nc.dram_tensor("waeT", [E, H], ATT_DT, kind="ExternalInput").ap(),
        "encT":   nc.dram_tensor("encT", [E, SC], ATT_DT, kind="ExternalInput").ap(),
        "encN":   nc.dram_tensor("encN", [SC, E], ATT_DT, kind="ExternalInput").ap(),
        "vk":     nc.dram_tensor("vk", [128, KH], f32, kind="ExternalInput").ap(),
        "wahcT":  nc.dram_tensor("wahcT", [E, H], WAHC_DT, kind="ExternalInput").ap(),
        "bah8":   nc.dram_tensor("bah8", [128, KH], f32, kind="ExternalInput").ap(),
        "woutP":  nc.dram_tensor("woutP", [8, 128, 4096], WO_DT, kind="ExternalInput").ap(),
        "bout":   nc.dram_tensor("bout", [128, 32], f32, kind="ExternalInput").ap(),
    }
    O = {
        "hout":   nc.dram_tensor("hout", [128, 1], f32, kind="ExternalOutput").ap(),
        "cout":   nc.dram_tensor("cout", [128, 1], f32, kind="ExternalOutput").ap(),
        "httout": nc.dram_tensor("httout", [128, KH], f32, kind="ExternalOutput").ap(),
        "wout":   nc.dram_tensor("wout", [128, 2], f32, kind="ExternalOutput").ap(),
        "lpout":  nc.dram_tensor("lpout", [128, 32], f32, kind="ExternalOutput").ap(),
    }
    with tile.TileContext(nc) as tc:
        with tc.tile_pool(name="wih", bufs=1) as wihp, \
             tc.tile_pool(name="watt", bufs=1) as wattp, \
             tc.tile_pool(name="wae", bufs=1) as waep, \
             tc.tile_pool(name="enct", bufs=1) as enctp, \
             tc.tile_pool(name="encn", bufs=1) as encnp, \
             tc.tile_pool(name="tz", bufs=1) as tzp, \
             tc.tile_pool(name="wahc", bufs=1) as wahcp, \
             tc.tile_pool(name="wo", bufs=wo_bufs) as wop, \
             tc.tile_pool(name="sm", bufs=2) as smp, \
             tc.tile_pool(name="const", bufs=1) as constp, \
             tc.tile_pool(name="dram", bufs=2, space="DRAM") as dram, \
             tc.tile_pool(name="psum", bufs=1, space="PSUM") as ps:
            ones = constp.tile([128, 1], f32, tag="ones")
            nc.vector.memset(ones[:], 1.0)
            one1 = constp.tile([1, 1], WIH_DT, tag="one1")
            nc.vector.memset(one1[:], 1.0)
            _G["ones"], _G["one1"] = ones, one1
            pools = (wihp, wattp, waep, enctp, encnp, tzp, wahcp, wop,
                     smp, dram, ps)
            for _ in range(n_iters):
                _emit_iter(nc, tc, I, O, pools, stop_after=stop_after)
    nc.compile()
    nc.m = get_hw_module(nc.m)
    return nc


# ---------------------------------------------------------------- host side
def shard_inputs(encoder_out, word_input, last_context, h0, c0,
                 emb, W_ih, W_hh, b_ih, b_hh,
                 W_attn, b_attn, v, W_ah, b_ah, W_out, b_out):
    f = np.float32
    enc = np.asarray(encoder_out, f)
    word = int(np.asarray(word_input).reshape(-1)[0])
    embedded = np.asarray(emb, f)[word]
    x = np.concatenate([np.asarray(last_context, f)[0], embedded])  # [H+E]
    wih_np, wsm_np = _np_dt(WIH_DT), _np_dt(WSM_DT)
    wo_np, wahc_np, att_np = _np_dt(WO_DT), _np_dt(WAHC_DT), _np_dt(ATT_DT)
    xk = np.ascontiguousarray(x.reshape(KX, 128).T).astype(wih_np)

    h0v = np.asarray(h0, f)[0, 0]
    c0v = np.asarray(c0, f)[0, 0]
    hbias = np.asarray(b_ih, f) + np.asarray(b_hh, f)
    if h0v.any():
        hbias = hbias + np.asarray(W_hh, f) @ h0v

    W_ih = np.asarray(W_ih, f)
    W_attn = np.asarray(W_attn, f)
    Wa_h, Wa_e = W_attn[:, :H], W_attn[:, H:]
    waeT = np.ascontiguousarray(Wa_e.T).astype(att_np)
    W_ah = np.asarray(W_ah, f)
    wahcT = np.ascontiguousarray(W_ah[:, :H].T).astype(wahc_np)  # [ctx, h]
    W_out = np.asarray(W_out, f)
    b_out = np.asarray(b_out, f)
    v = np.asarray(v, f)
    vk = np.ascontiguousarray(v.reshape(KH, 128).T)
    b_attn = np.asarray(b_attn, f)
    battn8 = np.ascontiguousarray(b_attn.reshape(KH, 128).T)     # [128, 8]
    bah8 = np.ascontiguousarray(np.asarray(b_ah, f).reshape(KH, 128).T)

    GORDER = (0, 1, 3, 2)  # i, f, o, g
    in_maps = []
    for k in range(NCORES):
        hs = slice(k * HC, (k + 1) * HC)
        rows = np.concatenate([np.arange(g * H + k * HC, g * H + (k + 1) * HC)
                               for g in GORDER])
        wihT = np.ascontiguousarray(W_ih[rows, :].T).astype(wih_np)
        hbiasr = np.ascontiguousarray(hbias[rows].reshape(1, 512)).astype(wih_np)
        c0_k = np.ascontiguousarray(c0v[hs].reshape(HC, 1))
        # column shards of Wa_h and W_ah[:, H:]: [128(k-chunk), 1024] each
        wattT = np.ascontiguousarray(
            np.concatenate([Wa_h[:, hs].T, W_ah[:, H + k * HC: H + (k + 1) * HC].T],
                           axis=1)).astype(wsm_np)               # [128, 2048]
        encT_k = np.ascontiguousarray(enc[k * SC:(k + 1) * SC, :].T).astype(att_np)
        encN_k = np.ascontiguousarray(enc[k * SC:(k + 1) * SC, :]).astype(att_np)
        wo_pad = np.zeros((VP, H), f)
        wo_pad[:VC] = W_out[k * VC:(k + 1) * VC, :]
        w4 = wo_pad.T.reshape(KH, 128, 32, 128)                  # [ec, p, mc, q]
        w4 = w4.transpose(2, 1, 0, 3)                            # [mc, p, ec, q]
        w4 = w4.reshape(8, 4, 128, KH, 128).transpose(0, 2, 1, 3, 4)
        woutP = np.ascontiguousarray(w4.reshape(8, 128, 4096)).astype(wo_np)
        bo_pad = np.full(VP, NEG_BIG, f)
        bo_pad[:VC] = b_out[k * VC:(k + 1) * VC]
        bout_k = np.ascontiguousarray(bo_pad.reshape(32, 128).T)
        in_maps.append({
            "xk": xk, "hbiasr": hbiasr, "c0k": c0_k, "wihT": wihT,
            "wattT": wattT, "battn8": battn8, "waeT": waeT,
            "encT": encT_k, "encN": encN_k, "vk": vk,
            "wahcT": wahcT, "bah8": bah8, "woutP": woutP, "bout": bout_k,
        })
    return in_maps


def assemble_outputs(results):
    f = np.float32
    ht = np.concatenate([results[k]["hout"].reshape(-1) for k in range(NCORES)])
    c = np.concatenate([results[k]["cout"].reshape(-1) for k in range(NCORES)])
    # httout is the FULL ht_tilda in K-layout [128,8] (identical on all cores)
    htt = results[0]["httout"].T.reshape(-1)
    w = np.concatenate(
        [results[k]["wout"].T.reshape(-1) for k in range(NCORES)])
    out = np.concatenate(
        [results[k]["lpout"].T.reshape(-1)[:VC] for k in range(NCORES)])
    return (out[None, :].astype(f),
            (ht[None, None, :].astype(f), c[None, None, :].astype(f)),
            htt[None, :].astype(f), w[None, :].astype(f))


_cached_nc = None


def kernel(**inputs):
    global _cached_nc
    if _cached_nc is None:
        _cached_nc = build_module(n_iters=1)
    in_maps = shard_inputs(**inputs)
    last_err = None
    for _attempt in range(3):
        try:
            res = bass_utils.run_bass_kernel_spmd(
                _cached_nc, in_maps, core_ids=list(range(NCORES)))
            return assemble_outputs(res.results)
        except Exception as e:  # transient device/tunnel hiccups
            last_err = e
            import time as _t
            _t.sleep(3.0)
    raise last_err


if __name__ == "__main__":
    import jax
    import reference
    with jax.default_device(jax.devices("cpu")[0]):
        inputs = {k: np.asarray(val) for k, val in reference.setup_inputs().items()}
        expected = jax.tree.map(np.asarray, reference.reference(**inputs))
    actual = kernel(**inputs)
    for (ep, e), (ap_, a) in zip(
            jax.tree_util.tree_leaves_with_path(expected),
            jax.tree_util.tree_leaves_with_path(actual)):
        e = np.asarray(e); a = np.asarray(a)
        rel = np.abs(a - e).max() / (np.abs(e).max() + 1e-12)
        print(f"{jax.tree_util.keystr(ep)}: rel={rel:.3e}")


# revision 14
# speedup vs baseline: 1.5127x; 1.5127x over previous
"""Trainium2 Bass kernel for a single-step attention decoder (DecoderRNN).

Computation (batch=1, single decode step):
  embedded = emb[word]                                  [E]
  x  = concat(last_context, embedded)                   [H+E]
  gates = W_ih @ x + b_ih + W_hh @ h0 + b_hh            [4H] (i,f,g,o)
  c  = sig(f)*c0 + sig(i)*tanh(g);  ht = sig(o)*tanh(c) [H]
  scores = tanh(enc @ Wa_e.T + (Wa_h @ ht + b_attn)) @ v  [S]
  w  = softmax(scores);  context = w @ enc              [E]
  ht_tilda = tanh(W_ah @ concat(context, ht) + b_ah)    [H]
  out = log_softmax(W_out @ ht_tilda + b_out)           [V]

Sharding across 8 NeuronCores, built to minimize the serial chain (three tiny
collectives total, no strided gathers):
  - LSTM rows hidden-sharded: core k produces ht_k, c_k [128].
  - Wa_h and W_ah[:,H:] are COLUMN-sharded: each core turns its own ht_k into
    full-length partial vectors u^k = Wa_h[:,k]@ht_k and w2^k = W_ah[:,H+k]@ht_k;
    AllReduce#1 sums both (payload [128,16], K-layout, contiguous).
  - encoder_out sequence-sharded for scores; exp(scores) partials give an
    unnormalized context partial [128,8] + local softmax normalizer;
    AllReduce#2 sums them (payload [128,9]).
  - ht_tilda computed FULLY on every core (replicated W_ah[:,:H] @ context
    + the AllReduced w2) -> feeds the vocab-sharded W_out matvec directly.
  - log_softmax normalizer via AllGather#3 of per-core sum(exp(logits)) (8 B).
Weight matvecs run on the PE with host-pre-transposed (optionally bf16)
weights; no max-subtraction needed (logits are O(1) by construction).
"""
import numpy as np

import concourse.bass as bass
import concourse.mybir as mybir
import concourse.tile as tile
from concourse import bacc, bass_utils
from concourse.bass_interp import get_hw_module

NCORES = 8
V, E, H, S = 32000, 1024, 1024, 2048
HC = H // NCORES      # 128  hidden chunk per core
SC = S // NCORES      # 256  sequence chunk per core
VC = V // NCORES      # 4000 vocab rows per core
VP = 4096             # padded vocab shard (32 tiles of 128)
KX = (E + H) // 128   # 16   contraction chunks for x
KH = H // 128         # 8    contraction chunks for ht-sized vectors
NEG_BIG = -1.0e5      # pad-row bias: exp() underflows to exactly 0

f32 = mybir.dt.float32
bf16 = mybir.dt.bfloat16
AF = mybir.ActivationFunctionType
ALU = mybir.AluOpType

# dtype config for the heavy weight streams (host casts to match)
WIH_DT = f32    # W_ih (gates)
WO_DT = bf16    # W_out (vocab projection)
WAHC_DT = f32   # W_ah[:, :H] (context part, replicated)
WSM_DT = f32    # Wa_h / W_ah[:,H:] column shards (small)
ATT_DT = f32    # Wa_e / enc (scores + context path stays f32 for w precision)

_G = {}  # build-scoped globals (ones tiles)


def _np_dt(dt):
    return np.float32 if dt == f32 else np.dtype("bfloat16")


# ---------------------------------------------------------------- device code
def _emit_iter(nc, tc, I, O, pools, stop_after=None):
    (wihp, wattp, waep, enctp, encnp, tzp, wahcp, wop, smp, dram, ps) = pools

    def dma(dst, src):
        nc.sync.dma_start(dst, src)

    ones = _G["ones"]        # [128,1] f32 1.0
    one1 = _G["one1"]        # [1,1] WIH_DT 1.0

    # ---- small constants ----
    xk = smp.tile([128, KX], WIH_DT, tag="xk")
    dma(xk[:], I["xk"])
    hbr = smp.tile([1, 512], WIH_DT, tag="hbr")
    dma(hbr[:], I["hbiasr"])
    c0 = smp.tile([128, 1], f32, tag="c0")
    dma(c0[:], I["c0k"])
    ub8 = smp.tile([128, KH], f32, tag="ub8")   # b_attn in K-layout
    dma(ub8[:], I["battn8"])
    bah8 = smp.tile([128, KH], f32, tag="bah8")
    dma(bah8[:], I["bah8"])
    vk = smp.tile([128, KH], f32, tag="vk")
    dma(vk[:], I["vk"])
    bout = smp.tile([128, 32], f32, tag="bout")
    dma(bout[:], I["bout"])

    # ================= Stage A: LSTM gates -> ht_k, c_k =================
    # gate columns: 0=i, 1=f, 2=o, 3=g (host reordered)
    wih = []
    for kc in range(KX):
        t = wihp.tile([128, 512], WIH_DT, tag=f"wih{kc % 8}", bufs=2)
        dma(t[:], I["wihT"][kc * 128:(kc + 1) * 128, :])
        wih.append(t)
    ps_g = ps.tile([128, 4], f32, tag="mm", bufs=3)
    for g in range(4):
        nc.tensor.matmul(ps_g[:, g:g + 1], hbr[:, g * 128:(g + 1) * 128],
                         one1[:], start=True, stop=False)
        for kc in range(KX):
            nc.tensor.matmul(ps_g[:, g:g + 1], wih[kc][:, g * 128:(g + 1) * 128],
                             xk[:, kc:kc + 1], start=False, stop=(kc == KX - 1))
    # sigmoid(x) = 0.5*tanh(0.5x)+0.5 for i,f,o; tanh for g — one table set.
    th3 = smp.tile([128, 3], f32, tag="th3")
    nc.scalar.activation(th3[:], ps_g[:, 0:3], AF.Tanh, scale=0.5)
    tan_g = smp.tile([128, 1], f32, tag="tang")
    nc.scalar.activation(tan_g[:], ps_g[:, 3:4], AF.Tanh)
    sg3 = smp.tile([128, 3], f32, tag="sg3")
    nc.vector.tensor_scalar(sg3[:], th3[:], 1.0, 0.5, ALU.add, ALU.mult)
    t1 = smp.tile([128, 1], f32, tag="sE")
    nc.vector.tensor_mul(t1[:], sg3[:, 1:2], c0[:])      # sig_f * c0
    t2 = smp.tile([128, 1], f32, tag="sF")
    nc.vector.tensor_mul(t2[:], sg3[:, 0:1], tan_g[:])   # sig_i * tanh(g)
    c_sb = smp.tile([128, 1], f32, tag="sG")
    nc.vector.tensor_add(c_sb[:], t1[:], t2[:])
    tan_c = smp.tile([128, 1], f32, tag="sH")
    nc.scalar.activation(tan_c[:], c_sb[:], AF.Tanh)
    ht_sb = smp.tile([128, 1], f32, tag="sI")
    nc.vector.tensor_mul(ht_sb[:], sg3[:, 2:3], tan_c[:])  # sig_o * tanh(c)
    dma(O["hout"], ht_sb[:])
    dma(O["cout"], c_sb[:])
    if stop_after == "A":
        return

    # ===== Stage B: partial u^k, w2^k (column-sharded matvecs) + AR1 =====
    htb = smp.tile([128, 1], WSM_DT, tag="htb")
    nc.vector.tensor_copy(htb[:], ht_sb[:])
    watt = wattp.tile([128, 2048], WSM_DT, tag="watt")   # [Wa_h col | W_ah col]
    dma(watt[:], I["wattT"])
    ps_uw = ps.tile([128, 16], f32, tag="mm", bufs=3)
    for mt in range(16):
        nc.tensor.matmul(ps_uw[:, mt:mt + 1], watt[:, mt * 128:(mt + 1) * 128],
                         htb[:], start=True, stop=True)
    uw_sb = smp.tile([128, 16], f32, tag="uwsb")
    nc.scalar.activation(uw_sb[:], ps_uw[:], AF.Copy)
    ar1_i = dram.tile([128, 16], f32, tag="ar1i")
    ar1_o = dram.tile([128, 16], f32, tag="ar1o")
    dma(ar1_i[:], uw_sb[:])
    nc.gpsimd.collective_compute("AllReduce", ALU.add,
                                 replica_groups=[list(range(NCORES))],
                                 ins=[ar1_i[:]], outs=[ar1_o[:]])
    uw8 = smp.tile([128, 16], f32, tag="uw8")
    dma(uw8[:], ar1_o[:])
    # u8 = u + b_attn (K-layout [128,8])
    u8 = smp.tile([128, KH], f32, tag="u8")
    nc.vector.tensor_add(u8[:], uw8[:, 0:KH], ub8[:])
    if stop_after == "B":
        dma(O["httout"], u8[:])
        return

    # ====== Stage C: Z.T = Wa_e @ enc_k.T ; scores_k = tanh(Z.T+u) @ v ==
    wae, enct = [], []
    for ec in range(KH):
        t = waep.tile([128, H], ATT_DT, tag=f"wae{ec}")
        dma(t[:], I["waeT"][ec * 128:(ec + 1) * 128, :])
        wae.append(t)
        t2_ = enctp.tile([128, SC], ATT_DT, tag=f"enct{ec}")
        dma(t2_[:], I["encT"][ec * 128:(ec + 1) * 128, :])
        enct.append(t2_)
    tz = []
    for hc in range(KH):
        ps_zt = ps.tile([128, SC], f32, tag="zt", bufs=4)
        for ec in range(KH):
            nc.tensor.matmul(ps_zt[:], wae[ec][:, hc * 128:(hc + 1) * 128],
                             enct[ec][:], start=(ec == 0), stop=(ec == KH - 1))
        t = tzp.tile([128, SC], f32, tag=f"tz{hc}")
        nc.scalar.activation(t[:], ps_zt[:], AF.Tanh, bias=u8[:, hc:hc + 1])
        tz.append(t)
    ps_s = ps.tile([128, 2], f32, tag="mm", bufs=3)
    for j in range(SC // 128):
        for hc in range(KH):
            nc.tensor.matmul(ps_s[:, j:j + 1], tz[hc][:, j * 128:(j + 1) * 128],
                             vk[:, hc:hc + 1], start=(hc == 0), stop=(hc == KH - 1))
    exp_sc = smp.tile([128, 2], f32, tag="expsc")
    zrow = smp.tile([128, 1], f32, tag="zrow")
    nc.scalar.activation(exp_sc[:], ps_s[:], AF.Exp, accum_out=zrow[:])

    # ===== Stage D: unnormalized context partial [128,8] + z ; AR2 =====
    encn = []
    for j in range(SC // 128):
        t = encnp.tile([128, E], ATT_DT, tag=f"encn{j}")
        dma(t[:], I["encN"][j * 128:(j + 1) * 128, :])
        encn.append(t)
    ps_ec = ps.tile([128, KH], f32, tag="mm", bufs=3)
    for mt in range(KH):
        for j in range(SC // 128):
            nc.tensor.matmul(ps_ec[:, mt:mt + 1],
                             encn[j][:, mt * 128:(mt + 1) * 128],
                             exp_sc[:, j:j + 1],
                             start=(j == 0), stop=(j == SC // 128 - 1))
    ps_z1 = ps.tile([1, 1], f32, tag="ec", bufs=1)
    nc.tensor.matmul(ps_z1[:], zrow[:], ones[:], start=True, stop=True)
    ecz = smp.tile([128, 9], f32, tag="ecz")
    nc.vector.memset(ecz[:, 8:9], 0.0)
    nc.scalar.activation(ecz[:, 0:8], ps_ec[:], AF.Copy)
    nc.scalar.activation(ecz[0:1, 8:9], ps_z1[:], AF.Copy)
    ar2_i = dram.tile([128, 9], f32, tag="ar2i")
    ar2_o = dram.tile([128, 9], f32, tag="ar2o")
    dma(ar2_i[:], ecz[:])
    nc.gpsimd.collective_compute("AllReduce", ALU.add,
                                 replica_groups=[list(range(NCORES))],
                                 ins=[ar2_i[:]], outs=[ar2_o[:]])
    ectx8 = smp.tile([128, KH], f32, tag="ectx8")
    dma(ectx8[:], ar2_o[:, 0:8])
    zb = smp.tile([128, 1], f32, tag="zb")
    dma(zb[:], ar2_o[0:1, 8:9].to_broadcast((128, 1)))
    rzb = smp.tile([128, 1], f32, tag="rzb")
    nc.vector.reciprocal(rzb[:], zb[:])
    ctx8 = smp.tile([128, KH], WAHC_DT, tag="ctx8")
    nc.vector.tensor_scalar_mul(ctx8[:], ectx8[:], rzb[:])
    w_sb = smp.tile([128, 2], f32, tag="wsb")
    nc.vector.tensor_scalar_mul(w_sb[:], exp_sc[:], rzb[:])
    dma(O["wout"], w_sb[:])
    if stop_after == "C":
        dma(O["httout"], ectx8[:])
        return

    # ===== Stage E: full ht_tilda = tanh(Wah_c@ctx + w2 + b_ah) [128,8] ==
    wahc = []
    for kc in range(KH):
        t = wahcp.tile([128, H], WAHC_DT, tag=f"wahc{kc}")
        dma(t[:], I["wahcT"][kc * 128:(kc + 1) * 128, :])
        wahc.append(t)
    ps_ht = ps.tile([128, KH], f32, tag="mm", bufs=3)
    for mt in range(KH):
        for kc in range(KH):
            nc.tensor.matmul(ps_ht[:, mt:mt + 1],
                             wahc[kc][:, mt * 128:(mt + 1) * 128],
                             ctx8[:, kc:kc + 1],
                             start=(kc == 0), stop=(kc == KH - 1))
    w2b = smp.tile([128, KH], f32, tag="w2b")
    nc.vector.tensor_add(w2b[:], uw8[:, KH:16], bah8[:])
    htsum = smp.tile([128, KH], f32, tag="htsum")
    nc.vector.tensor_add(htsum[:], ps_ht[:], w2b[:])
    htt8 = smp.tile([128, KH], WO_DT, tag="htt8")
    nc.scalar.activation(htt8[:], htsum[:], AF.Tanh)
    htt8f = smp.tile([128, KH], f32, tag="htt8f")
    nc.scalar.activation(htt8f[:], htsum[:], AF.Tanh)
    dma(O["httout"], htt8f[:])
    if stop_after == "E":
        return

    # ====== Stage F: logits_k = W_out[shard] @ ht_tilda ; log_softmax ===
    ps_lg = ps.tile([128, 32], f32, tag="mm", bufs=3)
    for g in range(8):
        wo = wop.tile([128, 4096], WO_DT, tag="wo")
        dma(wo[:], I["woutP"][g, :, :])
        for sub in range(4):
            mc = g * 4 + sub
            for ec in range(KH):
                nc.tensor.matmul(ps_lg[:, mc:mc + 1],
                                 wo[:, sub * 1024 + ec * 128: sub * 1024 + (ec + 1) * 128],
                                 htt8[:, ec:ec + 1],
                                 start=(ec == 0), stop=(ec == KH - 1))
    lg_sb = smp.tile([128, 32], f32, tag="lgsb")
    nc.vector.tensor_add(lg_sb[:], ps_lg[:], bout[:])
    exp2 = smp.tile([128, 32], f32, tag="exp2")
    z2row = smp.tile([128, 1], f32, tag="z2row")
    nc.scalar.activation(exp2[:], lg_sb[:], AF.Exp, accum_out=z2row[:])
    ps_z2 = ps.tile([1, 1], f32, tag="ec", bufs=1)
    nc.tensor.matmul(ps_z2[:], z2row[:], ones[:], start=True, stop=True)
    z2_sb = smp.tile([1, 1], f32, tag="z2sb")
    nc.scalar.activation(z2_sb[:], ps_z2[:], AF.Copy)
    ag_z_i = dram.tile([1, 1], f32, tag="agzi")
    ag_z_o = dram.tile([NCORES, 1], f32, tag="agzo")
    dma(ag_z_i[:], z2_sb[:])
    nc.gpsimd.collective_compute("AllGather", ALU.bypass,
                                 replica_groups=[list(range(NCORES))],
                                 ins=[ag_z_i[:]], outs=[ag_z_o[:]])
    z2all = smp.tile([128, NCORES], f32, tag="z2all")
    dma(z2all[:], ag_z_o[:].rearrange("(x j) y -> x (j y)", x=1).to_broadcast((128, NCORES)))
    z2g = smp.tile([128, 1], f32, tag="z2g")
    nc.vector.reduce_sum(z2g[:], z2all[:], axis=mybir.AxisListType.X)
    lnz = smp.tile([128, 1], f32, tag="lnz")
    nc.scalar.activation(lnz[:], z2g[:], AF.Ln)
    outp = smp.tile([128, 32], f32, tag="outp")
    nc.vector.tensor_scalar(outp[:], lg_sb[:], lnz[:], None, ALU.subtract)
    dma(O["lpout"], outp[:])


def build_module(n_iters=1, wo_bufs=7, stop_after=None):
    """Build + compile the SPMD module."""
    nc = bacc.Bacc("TRN2", target_bir_lowering=False, debug=False,
                   enable_asserts=False, num_devices=NCORES)
    I = {
        "xk":     nc.dram_tensor("xk", [128, KX], WIH_DT, kind="ExternalInput").ap(),
        "hbiasr": nc.dram_tensor("hbiasr", [1, 512], WIH_DT, kind="ExternalInput").ap(),
        "c0k":    nc.dram_tensor("c0k", [128, 1], f32, kind="ExternalInput").ap(),
        "wihT":   nc.dram_tensor("wihT", [E + H, 4 * 128], WIH_DT, kind="ExternalInput").ap(),
        "wattT":  nc.dram_tensor("wattT", [128, 2048], WSM_DT, kind="ExternalInput").ap(),
        "battn8": nc.dram_tensor("battn8", [128, KH], f32, kind="ExternalInput").ap(),
        "waeT":   nc.dram_tensor("waeT", [E, H], ATT_DT, kind="ExternalInput").ap(),
        "encT":   nc.dram_tensor("encT", [E, SC], ATT_DT, kind="ExternalInput").ap(),
        "encN":   nc.dram_tensor("encN", [SC, E], ATT_DT, kind="ExternalInput").ap(),
        "vk":     nc.dram_tensor("vk", [128, KH], f32, kind="ExternalInput").ap(),
        "wahcT":  nc.dram_tensor("wahcT", [E, H], WAHC_DT, kind="ExternalInput").ap(),
        "bah8":   nc.dram_tensor("bah8", [128, KH], f32, kind="ExternalInput").ap(),
        "woutP":  nc.dram_tensor("woutP", [8, 128, 4096], WO_DT, kind="ExternalInput").ap(),
        "bout":   nc.dram_tensor("bout", [128, 32], f32, kind="ExternalInput").ap(),
    }
    O = {
        "hout":   nc.dram_tensor("hout", [128, 1], f32, kind="ExternalOutput").ap(),
        "cout":   nc.dram_tensor("cout", [128, 1], f32, kind="ExternalOutput").ap(),
        "httout": nc.dram_tensor("httout", [128, KH], f32, kind="ExternalOutput").ap(),
        "wout":   nc.dram_tensor("wout", [128, 2], f32, kind="ExternalOutput").ap(),
        "lpout":  nc.dram_tensor("lpout", [128, 32], f32, kind="ExternalOutput").ap(),
    }
    with tile.TileContext(nc) as tc:
        with tc.tile_pool(name="wih", bufs=1) as wihp, \
             tc.tile_pool(name="watt", bufs=1) as wattp, \
             tc.tile_pool(name="wae", bufs=1) as waep, \
             tc.tile_pool(name="enct", bufs=1) as enctp, \
             tc.tile_pool(name="encn", bufs=1) as encnp, \
             tc.tile_pool(name="tz", bufs=1) as tzp, \
             tc.tile_pool(name="wahc", bufs=1) as wahcp, \
             tc.tile_pool(name="wo", bufs=wo_bufs) as wop, \
             tc.tile_pool(name="sm", bufs=2) as smp, \
             tc.tile_pool(name="const", bufs=1) as constp, \
             tc.tile_pool(name="dram", bufs=2, space="DRAM") as dram, \
             tc.tile_pool(name="psum", bufs=1, space="PSUM") as ps:
            ones = constp.tile([128, 1], f32, tag="ones")
            nc.vector.memset(ones[:], 1.0)
            one1 = constp.tile([1, 1], WIH_DT, tag="one1")
            nc.vector.memset(one1[:], 1.0)
            _G["ones"], _G["one1"] = ones, one1
            pools = (wihp, wattp, waep, enctp, encnp, tzp, wahcp, wop,
                     smp, dram, ps)
            for _ in range(n_iters):
                _emit_iter(nc, tc, I, O, pools, stop_after=stop_after)
    nc.compile()
    nc.m = get_hw_module(nc.m)
    return nc


# ---------------------------------------------------------------- host side
def shard_inputs(encoder_out, word_input, last_context, h0, c0,
                 emb, W_ih, W_hh, b_ih, b_hh,
                 W_attn, b_attn, v, W_ah, b_ah, W_out, b_out):
    f = np.float32
    enc = np.asarray(encoder_out, f)
    word = int(np.asarray(word_input).reshape(-1)[0])
    embedded = np.asarray(emb, f)[word]
    x = np.concatenate([np.asarray(last_context, f)[0], embedded])  # [H+E]
    wih_np, wsm_np = _np_dt(WIH_DT), _np_dt(WSM_DT)
    wo_np, wahc_np, att_np = _np_dt(WO_DT), _np_dt(WAHC_DT), _np_dt(ATT_DT)
    xk = np.ascontiguousarray(x.reshape(KX, 128).T).astype(wih_np)

    h0v = np.asarray(h0, f)[0, 0]
    c0v = np.asarray(c0, f)[0, 0]
    hbias = np.asarray(b_ih, f) + np.asarray(b_hh, f)
    if h0v.any():
        hbias = hbias + np.asarray(W_hh, f) @ h0v

    W_ih = np.asarray(W_ih, f)
    W_attn = np.asarray(W_attn, f)
    Wa_h, Wa_e = W_attn[:, :H], W_attn[:, H:]
    waeT = np.ascontiguousarray(Wa_e.T).astype(att_np)
    W_ah = np.asarray(W_ah, f)
    wahcT = np.ascontiguousarray(W_ah[:, :H].T).astype(wahc_np)  # [ctx, h]
    W_out = np.asarray(W_out, f)
    b_out = np.asarray(b_out, f)
    v = np.asarray(v, f)
    vk = np.ascontiguousarray(v.reshape(KH, 128).T)
    b_attn = np.asarray(b_attn, f)
    battn8 = np.ascontiguousarray(b_attn.reshape(KH, 128).T)     # [128, 8]
    bah8 = np.ascontiguousarray(np.asarray(b_ah, f).reshape(KH, 128).T)

    GORDER = (0, 1, 3, 2)  # i, f, o, g
    in_maps = []
    for k in range(NCORES):
        hs = slice(k * HC, (k + 1) * HC)
        rows = np.concatenate([np.arange(g * H + k * HC, g * H + (k + 1) * HC)
                               for g in GORDER])
        wihT = np.ascontiguousarray(W_ih[rows, :].T).astype(wih_np)
        hbiasr = np.ascontiguousarray(hbias[rows].reshape(1, 512)).astype(wih_np)
        c0_k = np.ascontiguousarray(c0v[hs].reshape(HC, 1))
        # column shards of Wa_h and W_ah[:, H:]: [128(k-chunk), 1024] each
        wattT = np.ascontiguousarray(
            np.concatenate([Wa_h[:, hs].T, W_ah[:, H + k * HC: H + (k + 1) * HC].T],
                           axis=1)).astype(wsm_np)               # [128, 2048]
        encT_k = np.ascontiguousarray(enc[k * SC:(k + 1) * SC, :].T).astype(att_np)
        encN_k = np.ascontiguousarray(enc[k * SC:(k + 1) * SC, :]).astype(att_np)
        wo_pad = np.zeros((VP, H), f)
        wo_pad[:VC] = W_out[k * VC:(k + 1) * VC, :]
        w4 = wo_pad.T.reshape(KH, 128, 32, 128)                  # [ec, p, mc, q]
        w4 = w4.transpose(2, 1, 0, 3)                            # [mc, p, ec, q]
        w4 = w4.reshape(8, 4, 128, KH, 128).transpose(0, 2, 1, 3, 4)
        woutP = np.ascontiguousarray(w4.reshape(8, 128, 4096)).astype(wo_np)
        bo_pad = np.full(VP, NEG_BIG, f)
        bo_pad[:VC] = b_out[k * VC:(k + 1) * VC]
        bout_k = np.ascontiguousarray(bo_pad.reshape(32, 128).T)
        in_maps.append({
            "xk": xk, "hbiasr": hbiasr, "c0k": c0_k, "wihT": wihT,
            "wattT": wattT, "battn8": battn8, "waeT": waeT,
            "encT": encT_k, "encN": encN_k, "vk": vk,
            "wahcT": wahcT, "bah8": bah8, "woutP": woutP, "bout": bout_k,
        })
    return in_maps


def assemble_outputs(results):
    f = np.float32
    ht = np.concatenate([results[k]["hout"].reshape(-1) for k in range(NCORES)])
    c = np.concatenate([results[k]["cout"].reshape(-1) for k in range(NCORES)])
    # httout is the FULL ht_tilda in K-layout [128,8] (identical on all cores)
    htt = results[0]["httout"].T.reshape(-1)
    w = np.concatenate(
        [results[k]["wout"].T.reshape(-1) for k in range(NCORES)])
    out = np.concatenate(
        [results[k]["lpout"].T.reshape(-1)[:VC] for k in range(NCORES)])
    return (out[None, :].astype(f),
            (ht[None, None, :].astype(f), c[None, None, :].astype(f)),
            htt[None, :].astype(f), w[None, :].astype(f))


_cached_nc = None


def kernel(**inputs):
    global _cached_nc
    if _cached_nc is None:
        _cached_nc = build_module(n_iters=1)
    in_maps = shard_inputs(**inputs)
    last_err = None
    for _attempt in range(3):
        try:
            res = bass_utils.run_bass_kernel_spmd(
                _cached_nc, in_maps, core_ids=list(range(NCORES)))
            return assemble_outputs(res.results)
        except Exception as e:  # transient device/tunnel hiccups
            last_err = e
            import time as _t
            _t.sleep(3.0)
    raise last_err


if __name__ == "__main__":
    import jax
    import reference
    with jax.default_device(jax.devices("cpu")[0]):
        inputs = {k: np.asarray(val) for k, val in reference.setup_inputs().items()}
        expected = jax.tree.map(np.asarray, reference.reference(**inputs))
    actual = kernel(**inputs)
    for (ep, e), (ap_, a) in zip(
            jax.tree_util.tree_leaves_with_path(expected),
            jax.tree_util.tree_leaves_with_path(actual)):
        e = np.asarray(e); a = np.asarray(a)
        rel = np.abs(a - e).max() / (np.abs(e).max() + 1e-12)
        print(f"{jax.tree_util.keystr(ep)}: rel={rel:.3e}")
